# revision 1
# baseline (speedup 1.0000x reference)
"""Trainium2 Bass kernel for nn_EnhancedDualRetriever (retrieval_knn).

Strategy (M-sharded retrieval across 8 cores):
 - Host (numpy, f64): tiny classifier + Bayesian changepoint + per-scale
   encoders -> queries q[S,B,D] and per-sample retrieval mode.
 - Device (SPMD, 8 cores, each owns M/8 = 12500 memory-bank rows per scale):
   bf16 matmul of augmented queries against augmented transposed keys
   (2 extra contraction rows implement the label/mode mask as -1e30 bias),
   col-tiled pair of matmuls fills a [128, 500] PSUM bank (two 500-column
   m-tiles stacked on the partition axis), DVE chunk-max (chunk=25) reduces
   each bank, then one Max8 + MaxIndex per scale yields the top-8 chunk
   maxima and their chunk indices per (b, half).
 - Host: merges the 8 cores' chunk candidates, exactly rescores candidate
   rows in f64 (guaranteed superset of the true top-5 per the chunk-max
   dominance argument), then reproduces the reference's softmax fusion.

Self-contained: hardcodes all shapes from the problem spec.
"""

import numpy as np
import ml_dtypes

# ---- problem dimensions (hardcoded per spec) ----
S, M, D, P_LEN = 4, 100000, 64, 96
B, T, NFEAT = 64, 512, 7
N_CORES = 8
M_LOC = M // N_CORES          # 12500
TILE = 1000                   # columns per kmat tile (two 500-col matmuls)
NT = 13                       # tiles per (s, core); last 500 cols are padding
CHUNK = 25                    # chunk-max granularity
RED_W = (TILE // 2) // CHUNK  # 20 chunks per bank half
TOPK = 5
MIN_SEG = 16
DS_RATES = [1, 2, 4, 8]
LN_EPS = 1e-5
NEG = -1.0e30

_BF16 = ml_dtypes.bfloat16


# --------------------------------------------------------------------------
# host-side math (numpy mirror of the reference's small parts, f64)
# --------------------------------------------------------------------------

def _host_small_parts(x, cls_w, cls_b, prior_mean, prior_var, noise_var,
                      enc_W, enc_b, ln_g, ln_b):
    dt = np.float64
    x = x.astype(dt)
    feats = np.stack([
        x.mean(axis=1).mean(axis=-1),
        np.clip(x.std(axis=1, ddof=1).mean(axis=-1), 1e-6, None),
        x.max(axis=1).mean(axis=-1),
        x.min(axis=1).mean(axis=-1),
        (x[:, -1, :] - x[:, 0, :]).mean(axis=-1)], axis=-1)
    extreme_prob = 1.0 / (1.0 + np.exp(-(feats @ cls_w.astype(dt) + cls_b.astype(dt)[0])))

    xf = x.mean(axis=-1)
    pv = np.log1p(np.exp(prior_var.astype(dt)))[0]
    nv = np.log1p(np.exp(noise_var.astype(dt)))[0]
    pm = prior_mean.astype(dt)[0]

    z = np.zeros((B, 1), dt)
    s1 = np.concatenate([z, np.cumsum(xf, axis=1)], axis=1)
    s2 = np.concatenate([z, np.cumsum(xf * xf, axis=1)], axis=1)
    pos = np.arange(MIN_SEG, T - MIN_SEG)
    nl = pos.astype(dt)
    nr = T - nl

    def stats(ssum, ssq, n):
        m = ssum / n
        v = np.clip((ssq - n * m * m) / np.maximum(n - 1.0, 1.0), 1e-8, None)
        return m, v

    def log_marginal(n, mean, var):
        post_var = 1.0 / (1.0 / pv + n / nv)
        post_mean = post_var * (pm / pv + n * mean / nv)
        return (-n / 2.0 * np.log(2.0 * np.pi * nv)
                + 0.5 * np.log(post_var / pv)
                - 0.5 * (n * var / nv + mean * mean * n / nv
                         - post_mean * post_mean / post_var + pm * pm / pv))

    ml_, vl = stats(s1[:, pos], s2[:, pos], nl)
    mr, vr = stats(s1[:, -1:] - s1[:, pos], s2[:, -1:] - s2[:, pos], nr)
    mw, vw = stats(s1[:, -1], s2[:, -1], float(T))
    bf = (log_marginal(nl, ml_, vl) + log_marginal(nr, mr, vr)
          - log_marginal(float(T), mw, vw)[:, None])
    bfmax = bf.max(axis=1)
    w = np.exp(bf - bfmax[:, None])
    w = w / w.sum(axis=1, keepdims=True)
    mask = (pos > int(T * 0.8)).astype(dt)
    near_end = (1.0 / (1.0 + np.exp(-bfmax))) * (w * mask).sum(axis=1)
    mode = np.where(near_end > 0.5, 2, np.where(extreme_prob > 0.5, 1, 0)).astype(np.int32)

    qs = []
    for ds in DS_RATES:
        if ds > 1:
            tT = T // ds * ds
            xd = x[:, :tT, :].reshape(B, tT // ds, ds, NFEAT).mean(axis=2)
        else:
            xd = x
        xfl = xd.mean(axis=-1)
        mean = xfl.mean(axis=1)
        std = np.clip(xfl.std(axis=1, ddof=1), 1e-6, None)
        stats5 = np.stack([mean, std, xfl.max(axis=1), xfl.min(axis=1),
                           xfl[:, -1] - xfl[:, 0]], axis=-1)
        h = stats5 @ enc_W.astype(dt) + enc_b.astype(dt)
        mu = h.mean(axis=-1, keepdims=True)
        var = h.var(axis=-1, keepdims=True)
        h = (h - mu) / np.sqrt(var + LN_EPS) * ln_g.astype(dt) + ln_b.astype(dt)
        qs.append(h / np.linalg.norm(h, axis=-1, keepdims=True))
    q = np.stack(qs)  # [S,B,D] f64
    return q, mode


# --------------------------------------------------------------------------
# device kernel (built & compiled once per process)
# --------------------------------------------------------------------------

_DEVICE = {}


def _build_device():
    if "nc" in _DEVICE:
        return _DEVICE["nc"]
    import concourse.mybir as mybir
    from concourse import bacc
    from concourse.tile import TileContext

    nc = bacc.Bacc()
    kq = nc.dram_tensor("kq", [S, 66, 128], mybir.dt.bfloat16, kind="ExternalInput")
    km = nc.dram_tensor("km", [S, NT, 66, TILE], mybir.dt.bfloat16, kind="ExternalInput")
    vals = nc.dram_tensor("vals", [S, 128, 8], mybir.dt.float32, kind="ExternalOutput")
    idx = nc.dram_tensor("idx", [S, 128, 8], mybir.dt.uint32, kind="ExternalOutput")

    with TileContext(nc) as tc:
        with (
            tc.tile_pool(name="kqp", bufs=1) as kqp,
            tc.tile_pool(name="kmp", bufs=4) as kmp,
            tc.tile_pool(name="psp", bufs=3, space="PSUM") as psp,
            tc.tile_pool(name="redp", bufs=2) as redp,
            tc.tile_pool(name="outp", bufs=2) as outp,
        ):
            kq_t = kqp.tile([66, S, 128], mybir.dt.bfloat16)
            nc.sync.dma_start(kq_t, kq[:].rearrange("s k m -> k s m"))

            for s in range(S):
                red = redp.tile([128, NT * RED_W], mybir.dt.float32, tag="red")
                # 6 pair-of-banks psum tiles (4 matmuls each) + 1 single
                for tp in range(7):
                    ps = psp.tile([128, 1024], mybir.dt.float32, tag="ps")
                    n_banks = 2 if tp < 6 else 1
                    for bank in range(n_banks):
                        t = 2 * tp + bank
                        kt = kmp.tile([66, TILE], mybir.dt.bfloat16, tag="kt")
                        nc.sync.dma_start(kt, km[s, t])
                        off = bank * 512
                        nc.tensor.matmul(ps[0:64, off:off + 500],
                                         lhsT=kq_t[:, s, 0:64],
                                         rhs=kt[:, 0:500], start=True, stop=True)
                        nc.tensor.matmul(ps[64:128, off:off + 500],
                                         lhsT=kq_t[:, s, 64:128],
                                         rhs=kt[:, 500:1000], start=True, stop=True)
                    # chunk-max over this psum tile -> red slice
                    in_ap = (ps.rearrange("p (k x) -> p k x", k=2)[:, 0:n_banks, 0:500]
                             .rearrange("p k (c w) -> p k c w", w=CHUNK))
                    out_ap = (red[:, 2 * tp * RED_W:(2 * tp + n_banks) * RED_W]
                              .rearrange("p (k c) -> p k c", k=n_banks))
                    nc.vector.reduce_max(out=out_ap, in_=in_ap,
                                         axis=mybir.AxisListType.X)
                v8 = outp.tile([128, 8], mybir.dt.float32, tag="v8")
                i8 = outp.tile([128, 8], mybir.dt.uint32, tag="i8")
                nc.vector.max(out=v8, in_=red)
                nc.vector.max_index(out=i8, in_max=v8, in_values=red)
                nc.sync.dma_start(vals[s], v8)
                nc.sync.dma_start(idx[s], i8)

    nc.compile()
    _DEVICE["nc"] = nc
    return nc


# --------------------------------------------------------------------------
# host orchestration
# --------------------------------------------------------------------------

def _prepare_device_inputs(q, mode, keys, labels):
    """q [S,B,D] f64, mode [B] -> kq bf16 [S,66,128]; per-core km bf16."""
    modeA = (mode == 1).astype(np.float32)
    modeB = (mode == 2).astype(np.float32)
    # stationary: [66, 64] = [q_s.T; modeA; modeB], duplicated to 128 cols
    kq = np.empty((S, 66, 128), dtype=_BF16)
    for s in range(S):
        qa = np.concatenate([q[s].astype(np.float32).T,
                             modeA[None, :], modeB[None, :]], axis=0)  # [66,64]
        kq[s] = np.concatenate([qa, qa], axis=1).astype(_BF16)

    kT = np.ascontiguousarray(keys.transpose(0, 2, 1)).astype(_BF16)  # [S,64,M]
    m1 = np.where(labels != 1, np.float32(NEG), np.float32(0.0)).astype(_BF16)  # [S,M]
    m2 = np.where(labels != 2, np.float32(NEG), np.float32(0.0)).astype(_BF16)

    in_maps = []
    for c in range(N_CORES):
        sl = slice(c * M_LOC, (c + 1) * M_LOC)
        km = np.zeros((S, 66, NT * TILE), dtype=_BF16)
        km[:, 0:64, 0:M_LOC] = kT[:, :, sl]
        km[:, 64, 0:M_LOC] = m1[:, sl]
        km[:, 65, 0:M_LOC] = m2[:, sl]
        km[:, 64:66, M_LOC:] = _BF16(NEG)  # padding never wins a chunk
        km = np.ascontiguousarray(
            km.reshape(S, 66, NT, TILE).transpose(0, 2, 1, 3))  # [S,NT,66,TILE]
        in_maps.append({"kq": kq, "km": km})
    return in_maps


def _merge_and_rescore(results, q, mode, keys, labels):
    """Merge per-core top-8 chunk maxima, exactly rescore candidates in f64."""
    # collect [S, B, n_cand] (value, m_start) across cores/halves
    vals5 = np.zeros((S, B, TOPK), np.float32)
    idx5 = np.zeros((S, B, TOPK), np.int64)

    # candidate chunk starts + values per (s,b): from each core, partitions
    # p<64 are b=p covering tile-half 0, p>=64 cover half 1.
    cand_vals = np.full((S, B, N_CORES * 2 * 8), -np.inf, np.float64)
    cand_start = np.zeros((S, B, N_CORES * 2 * 8), np.int64)
    for c, res in enumerate(results):
        v = res["vals"].astype(np.float64)   # [S,128,8]
        ix = res["idx"].astype(np.int64)     # [S,128,8]
        t = ix // RED_W
        cc = ix % RED_W
        mstart = c * M_LOC + t * TILE + cc * CHUNK          # half 0 (p<64)
        mstart2 = mstart + 500                               # half 1 (p>=64)
        j0 = c * 16
        cand_vals[:, :, j0:j0 + 8] = v[:, 0:64, :]
        cand_start[:, :, j0:j0 + 8] = mstart[:, 0:64, :]
        cand_vals[:, :, j0 + 8:j0 + 16] = v[:, 64:128, :]
        cand_start[:, :, j0 + 8:j0 + 16] = mstart2[:, 64:128, :]

    DELTA = 0.02
    keys64 = keys.astype(np.float64)
    for s in range(S):
        for b in range(B):
            cv = cand_vals[s, b]
            cs = cand_start[s, b]
            t5 = np.partition(cv, -5)[-5]
            keep = cs[cv >= t5 - DELTA]
            cand = (keep[:, None] + np.arange(CHUNK)[None, :]).ravel()
            cand = np.unique(cand[cand < M])
            # drop padding ranges (m beyond the core's real 12500)
            cand = cand[(cand % M_LOC < M_LOC)]
            esims = keys64[s, cand] @ q[s, b]
            if mode[b] != 0:
                esims = np.where(labels[s, cand] == mode[b], esims, -np.inf)
            if (np.isfinite(esims).sum()) < TOPK:
                # extremely defensive fallback: brute force this row
                esims = keys64[s] @ q[s, b]
                if mode[b] != 0:
                    esims = np.where(labels[s] == mode[b], esims, -np.inf)
                cand = np.arange(M)
            order = np.argsort(-esims, kind="stable")[:TOPK]
            vals5[s, b] = esims[order].astype(np.float32)
            idx5[s, b] = cand[order]
    return vals5, idx5


def kernel(x, keys, values, labels, thresholds, cls_w, cls_b,
           prior_mean, prior_var, noise_var, enc_W, enc_b, ln_g, ln_b):
    from concourse.bass_utils import run_bass_kernel_spmd

    x = np.asarray(x)
    keys = np.asarray(keys, dtype=np.float32)
    values = np.asarray(values, dtype=np.float32)
    labels = np.asarray(labels).astype(np.int32)
    thresholds = np.asarray(thresholds, dtype=np.float32)

    q, mode = _host_small_parts(
        np.asarray(x, np.float32), np.asarray(cls_w, np.float32),
        np.asarray(cls_b, np.float32), np.asarray(prior_mean, np.float32),
        np.asarray(prior_var, np.float32), np.asarray(noise_var, np.float32),
        np.asarray(enc_W, np.float32), np.asarray(enc_b, np.float32),
        np.asarray(ln_g, np.float32), np.asarray(ln_b, np.float32))

    nc = _build_device()
    in_maps = _prepare_device_inputs(q, mode, keys, labels)
    res = run_bass_kernel_spmd(nc, in_maps, core_ids=list(range(N_CORES)))
    vals5, idx5 = _merge_and_rescore(res.results, q, mode, keys, labels)

    # final fusion, mirroring the reference's f32 ops
    w = np.exp(vals5 - vals5.max(axis=2, keepdims=True))
    w = (w / w.sum(axis=2, keepdims=True)).astype(np.float32)
    gathered = values[np.arange(S)[:, None, None], idx5]          # [S,B,5,P]
    retr = np.einsum("sbk,sbkp->sbp", w, gathered).astype(np.float32)
    top1 = vals5[:, :, 0]                                          # [S,B]
    pvdr = (1.0 / (1.0 + np.exp(-(top1 - thresholds[:, None])))).astype(np.float32)
    sw = np.exp(top1 - top1.max(axis=0, keepdims=True))
    sw = (sw / sw.sum(axis=0, keepdims=True)).astype(np.float32)
    fused = np.einsum("sb,sbp->bp", sw, retr).astype(np.float32)
    out = np.array(np.broadcast_to(fused[:, :, None], (B, P_LEN, NFEAT)),
                   dtype=np.float32)
    return out, np.ascontiguousarray(pvdr.T)


# revision 3
# speedup vs baseline: 1861.8215x; 1861.8215x over previous
"""Trainium2 Bass kernel for nn_EnhancedDualRetriever (retrieval_knn).

Strategy (M-sharded retrieval across 8 cores):
 - Host (numpy, f64): tiny classifier + Bayesian changepoint + per-scale
   encoders -> queries q[S,B,D] and per-sample retrieval mode.
 - Device (SPMD, 8 cores, each owns M/8 = 12500 memory-bank rows per scale):
   bf16 matmul of augmented queries against augmented transposed keys
   (2 extra contraction rows implement the label/mode mask as -1e30 bias),
   col-tiled pairs of matmuls fill [128, 500] PSUM banks (two 500-column
   m-tiles stacked on the partition axis), DVE chunk-max (chunk=25) reduces
   4-bank PSUM quads, then one Max8 + MaxIndex per scale yields the top-8
   chunk maxima and their chunk indices per (b, half).
 - Host: merges the 8 cores' chunk candidates, exactly rescores candidate
   rows in f64 (guaranteed superset of the true top-5 per the chunk-max
   dominance argument), then reproduces the reference's softmax fusion.

Self-contained: hardcodes all shapes from the problem spec.
"""

import numpy as np
import ml_dtypes

# ---- problem dimensions (hardcoded per spec) ----
S, M, D, P_LEN = 4, 100000, 64, 96
B, T, NFEAT = 64, 512, 7
N_CORES = 8
M_LOC = M // N_CORES          # 12500
KTILE = 2000                  # columns per kmat tile (4 matmuls of 500)
NKT = 7                       # kmat tiles per (s, core); 12500 -> 14000 padded
CHUNK = 25                    # chunk-max granularity
RED_W = 20                    # chunks per 500-col bank half
RED_S = (NKT * KTILE // 2) // CHUNK  # 280 reduced cols per scale
TOPK = 5
MIN_SEG = 16
DS_RATES = [1, 2, 4, 8]
LN_EPS = 1e-5
NEG = -1.0e30

_BF16 = ml_dtypes.bfloat16


# --------------------------------------------------------------------------
# host-side math (numpy mirror of the reference's small parts, f64)
# --------------------------------------------------------------------------

def _host_small_parts(x, cls_w, cls_b, prior_mean, prior_var, noise_var,
                      enc_W, enc_b, ln_g, ln_b):
    dt = np.float64
    x = x.astype(dt)
    feats = np.stack([
        x.mean(axis=1).mean(axis=-1),
        np.clip(x.std(axis=1, ddof=1).mean(axis=-1), 1e-6, None),
        x.max(axis=1).mean(axis=-1),
        x.min(axis=1).mean(axis=-1),
        (x[:, -1, :] - x[:, 0, :]).mean(axis=-1)], axis=-1)
    extreme_prob = 1.0 / (1.0 + np.exp(-(feats @ cls_w.astype(dt) + cls_b.astype(dt)[0])))

    xf = x.mean(axis=-1)
    pv = np.log1p(np.exp(prior_var.astype(dt)))[0]
    nv = np.log1p(np.exp(noise_var.astype(dt)))[0]
    pm = prior_mean.astype(dt)[0]

    z = np.zeros((B, 1), dt)
    s1 = np.concatenate([z, np.cumsum(xf, axis=1)], axis=1)
    s2 = np.concatenate([z, np.cumsum(xf * xf, axis=1)], axis=1)
    pos = np.arange(MIN_SEG, T - MIN_SEG)
    nl = pos.astype(dt)
    nr = T - nl

    def stats(ssum, ssq, n):
        m = ssum / n
        v = np.clip((ssq - n * m * m) / np.maximum(n - 1.0, 1.0), 1e-8, None)
        return m, v

    def log_marginal(n, mean, var):
        post_var = 1.0 / (1.0 / pv + n / nv)
        post_mean = post_var * (pm / pv + n * mean / nv)
        return (-n / 2.0 * np.log(2.0 * np.pi * nv)
                + 0.5 * np.log(post_var / pv)
                - 0.5 * (n * var / nv + mean * mean * n / nv
                         - post_mean * post_mean / post_var + pm * pm / pv))

    ml_, vl = stats(s1[:, pos], s2[:, pos], nl)
    mr, vr = stats(s1[:, -1:] - s1[:, pos], s2[:, -1:] - s2[:, pos], nr)
    mw, vw = stats(s1[:, -1], s2[:, -1], float(T))
    bf = (log_marginal(nl, ml_, vl) + log_marginal(nr, mr, vr)
          - log_marginal(float(T), mw, vw)[:, None])
    bfmax = bf.max(axis=1)
    w = np.exp(bf - bfmax[:, None])
    w = w / w.sum(axis=1, keepdims=True)
    mask = (pos > int(T * 0.8)).astype(dt)
    near_end = (1.0 / (1.0 + np.exp(-bfmax))) * (w * mask).sum(axis=1)
    mode = np.where(near_end > 0.5, 2, np.where(extreme_prob > 0.5, 1, 0)).astype(np.int32)

    qs = []
    for ds in DS_RATES:
        if ds > 1:
            tT = T // ds * ds
            xd = x[:, :tT, :].reshape(B, tT // ds, ds, NFEAT).mean(axis=2)
        else:
            xd = x
        xfl = xd.mean(axis=-1)
        mean = xfl.mean(axis=1)
        std = np.clip(xfl.std(axis=1, ddof=1), 1e-6, None)
        stats5 = np.stack([mean, std, xfl.max(axis=1), xfl.min(axis=1),
                           xfl[:, -1] - xfl[:, 0]], axis=-1)
        h = stats5 @ enc_W.astype(dt) + enc_b.astype(dt)
        mu = h.mean(axis=-1, keepdims=True)
        var = h.var(axis=-1, keepdims=True)
        h = (h - mu) / np.sqrt(var + LN_EPS) * ln_g.astype(dt) + ln_b.astype(dt)
        qs.append(h / np.linalg.norm(h, axis=-1, keepdims=True))
    q = np.stack(qs)  # [S,B,D] f64
    return q, mode


# --------------------------------------------------------------------------
# device kernel (built & compiled once per process)
# --------------------------------------------------------------------------

_DEVICE = {}


def _build_device():
    if "nc" in _DEVICE:
        return _DEVICE["nc"]
    import concourse.mybir as mybir
    from concourse import bacc
    from concourse.tile import TileContext

    nc = bacc.Bacc()
    kq = nc.dram_tensor("kq", [66, S, 128], mybir.dt.bfloat16, kind="ExternalInput")
    km = nc.dram_tensor("km", [S, NKT, 66, KTILE], mybir.dt.bfloat16,
                        kind="ExternalInput")
    vals = nc.dram_tensor("vals", [S, 128, 8], mybir.dt.float32, kind="ExternalOutput")
    idx = nc.dram_tensor("idx", [S, 128, 8], mybir.dt.uint32, kind="ExternalOutput")

    with TileContext(nc) as tc:
        with (
            tc.tile_pool(name="kqp", bufs=1) as kqp,
            tc.tile_pool(name="kmp", bufs=6) as kmp,
            tc.tile_pool(name="psp", bufs=2, space="PSUM") as psp,
            tc.tile_pool(name="redp", bufs=2) as redp,
            tc.tile_pool(name="outp", bufs=2) as outp,
        ):
            kq_t = kqp.tile([66, S, 128], mybir.dt.bfloat16)
            nc.sync.dma_start(kq_t, kq[:])

            n_dma = 0
            for s in range(S):
                red = redp.tile([128, RED_S], mybir.dt.float32, tag="red")
                # 3 full psum quads (kt tiles 0-5) + 1 half quad (kt tile 6)
                for quad in range(4):
                    ps = psp.tile([128, 2048], mybir.dt.float32, tag="ps")
                    n_kt = 2 if quad < 3 else 1
                    for i in range(n_kt):
                        t = 2 * quad + i
                        kt = kmp.tile([66, KTILE], mybir.dt.bfloat16, tag="kt")
                        # alternate DMA issue between SP and ACT sequencers
                        dma_eng = nc.sync if (n_dma % 2 == 0) else nc.scalar
                        dma_eng.dma_start(kt, km[s, t])
                        n_dma += 1
                        for g in range(2):  # 1000-col group -> one psum bank
                            off = (2 * i + g) * 512
                            col = 1000 * g
                            nc.tensor.matmul(
                                ps[0:64, off:off + 500],
                                lhsT=kq_t[:, s, 0:64],
                                rhs=kt[:, col:col + 500],
                                start=True, stop=True)
                            nc.tensor.matmul(
                                ps[64:128, off:off + 500],
                                lhsT=kq_t[:, s, 64:128],
                                rhs=kt[:, col + 500:col + 1000],
                                start=True, stop=True)
                    n_banks = 2 * n_kt
                    in_ap = (ps.rearrange("p (k x) -> p k x", k=4)[:, 0:n_banks, 0:500]
                             .rearrange("p k (c w) -> p k c w", w=CHUNK))
                    out_ap = (red[:, quad * 4 * RED_W:
                                  quad * 4 * RED_W + n_banks * RED_W]
                              .rearrange("p (k c) -> p k c", k=n_banks))
                    nc.vector.reduce_max(out=out_ap, in_=in_ap,
                                         axis=mybir.AxisListType.X)
                v8 = outp.tile([128, 8], mybir.dt.float32, tag="v8")
                i8 = outp.tile([128, 8], mybir.dt.uint32, tag="i8")
                nc.vector.max(out=v8, in_=red)
                nc.vector.max_index(out=i8, in_max=v8, in_values=red)
                nc.sync.dma_start(vals[s], v8)
                nc.sync.dma_start(idx[s], i8)

    nc.compile()
    _DEVICE["nc"] = nc
    return nc


# --------------------------------------------------------------------------
# host orchestration
# --------------------------------------------------------------------------

def _prepare_device_inputs(q, mode, keys, labels):
    """q [S,B,D] f64, mode [B] -> kq bf16 [66,S,128]; per-core km bf16."""
    modeA = (mode == 1).astype(np.float32)
    modeB = (mode == 2).astype(np.float32)
    kq = np.empty((66, S, 128), dtype=_BF16)
    for s in range(S):
        qa = np.concatenate([q[s].astype(np.float32).T,
                             modeA[None, :], modeB[None, :]], axis=0)  # [66,64]
        kq[:, s, 0:64] = qa.astype(_BF16)
        kq[:, s, 64:128] = qa.astype(_BF16)

    kT = np.ascontiguousarray(keys.transpose(0, 2, 1)).astype(_BF16)  # [S,64,M]
    m1 = np.where(labels != 1, np.float32(NEG), np.float32(0.0)).astype(_BF16)
    m2 = np.where(labels != 2, np.float32(NEG), np.float32(0.0)).astype(_BF16)

    in_maps = []
    for c in range(N_CORES):
        sl = slice(c * M_LOC, (c + 1) * M_LOC)
        km = np.zeros((S, 66, NKT * KTILE), dtype=_BF16)
        km[:, 0:64, 0:M_LOC] = kT[:, :, sl]
        km[:, 64, 0:M_LOC] = m1[:, sl]
        km[:, 65, 0:M_LOC] = m2[:, sl]
        km[:, 64:66, M_LOC:] = _BF16(NEG)  # padding never wins a chunk
        km = np.ascontiguousarray(
            km.reshape(S, 66, NKT, KTILE).transpose(0, 2, 1, 3))  # [S,NKT,66,KTILE]
        in_maps.append({"kq": kq, "km": km})
    return in_maps


def _merge_and_rescore(results, q, mode, keys, labels):
    """Merge per-core top-8 chunk maxima, exactly rescore candidates in f64.

    Reduced-array index j in [0, 280) decodes as:
      quad = min(j // 80, 3); bank = (j - quad*80) // 20; cc = j % 20
      kt tile t = 2*quad + bank//2 ; group g = bank % 2
      m_local = t*2000 + g*1000 + half*500 + cc*25   (half: partition >= 64)
    """
    vals5 = np.zeros((S, B, TOPK), np.float32)
    idx5 = np.zeros((S, B, TOPK), np.int64)

    cand_vals = np.full((S, B, N_CORES * 2 * 8), -np.inf, np.float64)
    cand_start = np.zeros((S, B, N_CORES * 2 * 8), np.int64)
    for c, res in enumerate(results):
        v = res["vals"].astype(np.float64)   # [S,128,8]
        ix = res["idx"].astype(np.int64)     # [S,128,8]
        quad = np.minimum(ix // 80, 3)
        bank = (ix - quad * 80) // 20
        cc = ix % 20
        t = 2 * quad + bank // 2
        g = bank % 2
        # virtual index: core-major with the padded 14000 stride, so the
        # padding region can be excluded before mapping to global m
        vstart = c * (NKT * KTILE) + t * KTILE + g * 1000 + cc * CHUNK  # half 0
        j0 = c * 16
        cand_vals[:, :, j0:j0 + 8] = v[:, 0:64, :]
        cand_start[:, :, j0:j0 + 8] = vstart[:, 0:64, :]
        cand_vals[:, :, j0 + 8:j0 + 16] = v[:, 64:128, :]
        cand_start[:, :, j0 + 8:j0 + 16] = vstart[:, 64:128, :] + 500

    DELTA = 0.02
    VSTRIDE = NKT * KTILE  # 14000
    keys64 = keys.astype(np.float64)
    chunk_off = np.arange(CHUNK)
    for s in range(S):
        for b in range(B):
            cv = cand_vals[s, b]
            cs = cand_start[s, b]
            t5 = np.partition(cv, -5)[-5]
            keep = cs[cv >= t5 - DELTA]
            # chunk starts are 25-aligned; 12500 is a chunk boundary, so a
            # chunk is either fully real or fully padding
            keep = keep[keep % VSTRIDE < M_LOC]
            cand = (keep[:, None] + chunk_off[None, :]).ravel()
            cand = np.unique(cand // VSTRIDE * M_LOC + cand % VSTRIDE)
            esims = keys64[s, cand] @ q[s, b]
            if mode[b] != 0:
                esims = np.where(labels[s, cand] == mode[b], esims, -np.inf)
            if np.isfinite(esims).sum() < TOPK:
                esims = keys64[s] @ q[s, b]
                if mode[b] != 0:
                    esims = np.where(labels[s] == mode[b], esims, -np.inf)
                cand = np.arange(M)
            order = np.argsort(-esims, kind="stable")[:TOPK]
            vals5[s, b] = esims[order].astype(np.float32)
            idx5[s, b] = cand[order]
    return vals5, idx5


def kernel(x, keys, values, labels, thresholds, cls_w, cls_b,
           prior_mean, prior_var, noise_var, enc_W, enc_b, ln_g, ln_b):
    from concourse.bass_utils import run_bass_kernel_spmd

    x = np.asarray(x)
    keys = np.asarray(keys, dtype=np.float32)
    values = np.asarray(values, dtype=np.float32)
    labels = np.asarray(labels).astype(np.int32)
    thresholds = np.asarray(thresholds, dtype=np.float32)

    q, mode = _host_small_parts(
        np.asarray(x, np.float32), np.asarray(cls_w, np.float32),
        np.asarray(cls_b, np.float32), np.asarray(prior_mean, np.float32),
        np.asarray(prior_var, np.float32), np.asarray(noise_var, np.float32),
        np.asarray(enc_W, np.float32), np.asarray(enc_b, np.float32),
        np.asarray(ln_g, np.float32), np.asarray(ln_b, np.float32))

    nc = _build_device()
    in_maps = _prepare_device_inputs(q, mode, keys, labels)
    res = run_bass_kernel_spmd(nc, in_maps, core_ids=list(range(N_CORES)))
    vals5, idx5 = _merge_and_rescore(res.results, q, mode, keys, labels)

    # final fusion, mirroring the reference's f32 ops
    w = np.exp(vals5 - vals5.max(axis=2, keepdims=True))
    w = (w / w.sum(axis=2, keepdims=True)).astype(np.float32)
    gathered = values[np.arange(S)[:, None, None], idx5]          # [S,B,5,P]
    retr = np.einsum("sbk,sbkp->sbp", w, gathered).astype(np.float32)
    top1 = vals5[:, :, 0]                                          # [S,B]
    pvdr = (1.0 / (1.0 + np.exp(-(top1 - thresholds[:, None])))).astype(np.float32)
    sw = np.exp(top1 - top1.max(axis=0, keepdims=True))
    sw = (sw / sw.sum(axis=0, keepdims=True)).astype(np.float32)
    fused = np.einsum("sb,sbp->bp", sw, retr).astype(np.float32)
    out = np.array(np.broadcast_to(fused[:, :, None], (B, P_LEN, NFEAT)),
                   dtype=np.float32)
    return out, np.ascontiguousarray(pvdr.T)
